# revision 1
# baseline (speedup 1.0000x reference)
"""DGCNN (nn_DGCNNModule_16458314678665) Trainium2 Bass kernel.

Strategy (data-parallel over batch B=8, one point-cloud per NeuronCore):
  Per conv (dynamic-graph edge conv), per cloud [N=2048 points]:
    - pd = -||xi-xj||^2 computed as a single PE matmul with augmented
      operands:  pd = X^T.(2X) + [-1;-sq]^T.[sq;1]   (diag exactly 0)
    - exact top-20 neighbor selection per row via 3 rounds of the DVE
      max8 / max_index / match_replace trio (full-width rows)
    - edge conv decomposed (BN scale>0 and LeakyReLU are monotone):
        out = lrelu( max_k P[idx_k] + Q )
      with P = X Wn~^T (gathered via indirect DMA from DRAM),
      Q = X (Wc~-Wn~)^T + b~  (BN folded into weights on host)
    - PE transpose back to channel-major for the next conv's pd matmul
  Head: h5 = lrelu(bn(cat @ W5^T)) on PE with per-channel BN via the ACT
  engine's per-partition scale/bias; max+mean pooling along the free axis;
  3 tiny matvecs; biases folded host-side where possible.

kernel(**inputs) takes the FULL unsharded inputs and returns [8, 40] f32.
"""

import sys
import numpy as np

for _p in ("/opt/trn_rl_repo", "/root/.axon_site/_ro/trn_rl_repo"):
    if _p not in sys.path:
        sys.path.insert(0, _p)

import concourse.bass as bass
import concourse.bacc as bacc
import concourse.mybir as mybir
from concourse.tile import TileContext
from concourse.bass_utils import run_bass_kernel_spmd

F32 = mybir.dt.float32
F32R = mybir.dt.float32r
U32 = mybir.dt.uint32

N = 2048
NB = 16          # 128-point blocks
K = 20
BN_INV = 1.0 / float(np.sqrt(1.0 + 1e-5))
CONVS = [(3, 64), (64, 64), (64, 128), (128, 256)]  # (C_in, O)
NEG = -3.0e38

_CACHE = {}


def build_nc(debug_taps=False):
    nc = bacc.Bacc("TRN2", target_bir_lowering=False, debug=False, num_swdge_queues=4)
    AF = mybir.ActivationFunctionType

    dbg = {}
    if debug_taps:
        for nm, shp, dt in [("dbg_pd", [128, N], F32), ("dbg_idx", [128, 24], U32),
                            ("dbg_nbr", [128, K * 64], F32), ("dbg_red", [128, 64], F32),
                            ("dbg_q", [128, 64], F32), ("dbg_p0", [128, 64], F32),
                            ("dbg_sq", [1, N], F32)]:
            dbg[nm] = nc.declare_dram_parameter(nm, shp, dt, isOutput=True)

    # ---------------- DRAM parameters ----------------
    xT = nc.declare_dram_parameter("xT", [3, N], F32, isOutput=False)
    wn, wcn = [], []
    for li, (C, O) in enumerate(CONVS):
        wn.append(nc.declare_dram_parameter(f"wn{li}", [C, O], F32, isOutput=False))
        wcn.append(nc.declare_dram_parameter(f"wcn{li}", [C, O], F32, isOutput=False))
    w5 = nc.declare_dram_parameter("w5", [512, 1024], F32R, isOutput=False)
    s5 = nc.declare_dram_parameter("s5", [1024], F32, isOutput=False)
    b5 = nc.declare_dram_parameter("b5", [1024], F32, isOutput=False)
    wl1 = nc.declare_dram_parameter("wl1", [2048, 512], F32, isOutput=False)
    bias6 = nc.declare_dram_parameter("bias6", [512], F32, isOutput=False)
    wl2 = nc.declare_dram_parameter("wl2", [512, 256], F32, isOutput=False)
    bias2 = nc.declare_dram_parameter("bias2", [256], F32, isOutput=False)
    qb = [nc.declare_dram_parameter(f"qb{li}", [1, O], F32, isOutput=False)
          for li, (C, O) in enumerate(CONVS)]
    wl3 = nc.declare_dram_parameter("wl3", [256, 40], F32, isOutput=False)
    bias3 = nc.declare_dram_parameter("bias3", [40], F32, isOutput=False)
    ident = nc.declare_dram_parameter("ident", [128, 128], F32, isOutput=False)
    onesrow_d = nc.declare_dram_parameter("onesrow", [1, N], F32, isOutput=False)
    negonesrow_d = nc.declare_dram_parameter("negonesrow", [1, N], F32, isOutput=False)
    out = nc.declare_dram_parameter("out", [40], F32, isOutput=True)

    with TileContext(nc) as tc:
        from contextlib import ExitStack

        # internal DRAM staging for the P projections (gathered back by rows).
        # Must be tile-pool tiles so Tile tracks write->gather dependencies.
        dram_pool_stack = ExitStack()
        dp = dram_pool_stack.enter_context(tc.tile_pool(name="dramstage", bufs=1, space="DRAM"))
        p_dram = [dp.tile([N, O], F32, tag=f"p_stage{li}", name=f"p_stage{li}")
                  for li, (C, O) in enumerate(CONVS)]

        # ---------------- persistent pools ----------------
        persist_stack = ExitStack()
        pp = persist_stack.enter_context(tc.tile_pool(name="persist", bufs=1))
        if True:
            identsb = pp.tile([128, 128], F32, tag="ident")
            nc.sync.dma_start(out=identsb[:], in_=ident[:])
            xTsb = pp.tile([3, N], F32, tag="xT")
            nc.sync.dma_start(out=xTsb[:], in_=xT[:])
            # channel-major per-conv output features (cat^T as 5 k-tiles of
            # rows [x1(64); x2(64); x3(128); x4a(128); x4b(128)])
            CATROWS = [64, 64, 128, 128, 128]
            catT = [pp.tile([r, N], F32, tag=f"catT{i}", name=f"catT{i}")
                    for i, r in enumerate(CATROWS)]
            # small per-conv weights
            wnsb = []
            wcnsb = []
            qbsb = []
            for li, (C, O) in enumerate(CONVS):
                t1 = pp.tile([C, O], F32, tag=f"wn{li}", name=f"wnsb{li}")
                nc.sync.dma_start(out=t1[:], in_=wn[li][:])
                t2 = pp.tile([C, O], F32, tag=f"wcn{li}", name=f"wcnsb{li}")
                nc.sync.dma_start(out=t2[:], in_=wcn[li][:])
                t3 = pp.tile([1, O], F32, tag=f"qb{li}", name=f"qbsb{li}")
                nc.sync.dma_start(out=t3[:], in_=qb[li][:])
                wnsb.append(t1)
                wcnsb.append(t2)
                qbsb.append(t3)
            # pooled features (rhs k-tiles of the first head matmul)
            fmax = [pp.tile([128, 1], F32, tag=f"fmax{t}", name=f"fmax{t}") for t in range(8)]
            fsum = [pp.tile([128, 1], F32, tag=f"fsum{t}", name=f"fsum{t}") for t in range(8)]

        conv_stack = ExitStack()
        sc = conv_stack.enter_context(tc.tile_pool(name="conv_sbuf", bufs=1))
        scd = conv_stack.enter_context(tc.tile_pool(name="conv_dbl", bufs=2))
        psum = conv_stack.enter_context(tc.tile_pool(name="conv_psum", bufs=1, space="PSUM"))
        psum_sm = conv_stack.enter_context(tc.tile_pool(name="conv_psum_sm", bufs=2, space="PSUM"))

        curXT = xTsb[:]  # [C, N] channel-major current features
        for li, (C, O) in enumerate(CONVS):
            # ---------- stats & operands ----------
            # Augmented pd operands. For C <= 126 the matmul is one pass with
            # lhsT = Aaug = [X; -1; -sq], rhs = Baug = [2X; sq; 1]; for conv4
            # (C=128) the two extra rows go in a separate K=2 pass.
            folded = C + 2 <= 128
            ar = C if folded else 0  # aug row offset inside Aaug/Baug
            if folded:
                Aaug = sc.tile([C + 2, N], F32, tag="Aaug")
                Baug = sc.tile([C + 2, N], F32, tag="Baug")
                nc.scalar.activation(out=Aaug[0:C, :], in_=curXT, func=AF.Copy)
                nc.scalar.activation(out=Baug[0:C, :], in_=curXT, func=AF.Copy, scale=2.0)
                aux_a, aux_b = Aaug, Baug
            else:
                twoX = sc.tile([C, N], F32, tag="Baug")
                nc.scalar.activation(out=twoX[:], in_=curXT, func=AF.Copy, scale=2.0)
                aux_a = sc.tile([2, N], F32, tag="aux_a")   # [-1; -sq]
                aux_b = sc.tile([2, N], F32, tag="aux_b")   # [sq; 1]
            x2 = sc.tile([C, N], F32, tag="x2")
            nc.scalar.activation(out=x2[:], in_=curXT, func=AF.Square)
            onesC = sc.tile([C, 1], F32, tag="onesC")
            nc.gpsimd.memset(onesC[:], 1.0)
            sqrow = sc.tile([1, N], F32, tag="sqrow")
            negsq = sc.tile([1, N], F32, tag="negsq")
            for j in range(4):
                sqp = psum_sm.tile([1, 512], F32, tag="sq", bufs=1)
                nc.tensor.matmul(sqp[:], onesC[:], x2[:, j * 512:(j + 1) * 512],
                                 start=True, stop=True)
                nc.scalar.copy(sqrow[0:1, j * 512:(j + 1) * 512], sqp[:])
            nc.vector.tensor_scalar_mul(negsq[:], sqrow[:], -1.0)
            negones = sc.tile([1, N], F32, tag="negones")
            nc.sync.dma_start(out=negones[:], in_=negonesrow_d[:])
            # assemble aug rows via DMA (engines cannot write at partition>0)
            nc.sync.dma_start(out=aux_a[ar:ar + 1, :], in_=negonesrow_d[:])
            nc.sync.dma_start(out=aux_a[ar + 1:ar + 2, :], in_=negsq[:])
            nc.sync.dma_start(out=aux_b[ar:ar + 1, :], in_=sqrow[:])
            nc.sync.dma_start(out=aux_b[ar + 1:ar + 2, :], in_=onesrow_d[:])
            if debug_taps and li == 0:
                nc.sync.dma_start(out=dbg["dbg_sq"][:], in_=sqrow[:])

            # ---------- hoist pd block 0 so DVE starts while PE projects ----------
            def make_pd(m):
                blk = slice(m * 128, (m + 1) * 128)
                pdps = psum.tile([128, N], F32, tag="pd", name=f"pdps{li}_{m}")
                for j in range(4):
                    cols = slice(j * 512, (j + 1) * 512)
                    if folded:
                        nc.tensor.matmul(pdps[:, cols], Aaug[:, blk], Baug[:, cols],
                                         start=True, stop=True)
                    else:
                        nc.tensor.matmul(pdps[:, cols], curXT[:, blk], twoX[:, cols],
                                         start=True, stop=False)
                        nc.tensor.matmul(pdps[:, cols], aux_a[:, blk], aux_b[:, cols],
                                         start=False, stop=True)
                pdsb = scd.tile([128, N], F32, tag="pdsb", name=f"pdsb{li}_{m}")
                for j in range(4):
                    cols = slice(j * 512, (j + 1) * 512)
                    nc.scalar.copy(pdsb[:, cols], pdps[:, cols])
                return pdsb

            pd0 = make_pd(0)

            # ---------- projections P (to DRAM) and Q (kept) ----------
            qsb = sc.tile([128, NB * O], F32, tag="q")
            for m in range(NB):
                blk = slice(m * 128, (m + 1) * 128)
                pps = psum_sm.tile([128, O], F32, tag="proj")
                nc.tensor.matmul(pps[:], curXT[:, blk], wnsb[li][:], start=True, stop=True)
                ppt = scd.tile([128, O], F32, tag="ppt")
                nc.scalar.copy(ppt[:], pps[:])
                if debug_taps and li == 0 and m == 0:
                    nc.sync.dma_start(out=dbg["dbg_p0"][:], in_=ppt[:])
                nc.sync.dma_start(out=p_dram[li][m * 128:(m + 1) * 128, :], in_=ppt[:])
                qps = psum_sm.tile([128, O], F32, tag="proj")
                nc.tensor.matmul(qps[:], curXT[:, blk], wcnsb[li][:], start=True, stop=False)
                nc.tensor.matmul(qps[:], negones[0:1, blk], qbsb[li][:], start=False, stop=True)
                nc.scalar.copy(qsb[:, m * O:(m + 1) * O], qps[:])

            # ---------- per-block: pd, top-k, gather, k-max ----------
            for m in range(NB):
                blk = slice(m * 128, (m + 1) * 128)
                pdsb = pd0 if m == 0 else make_pd(m)
                if debug_taps and li == 0 and m == 0:
                    nc.sync.dma_start(out=dbg["dbg_pd"][:], in_=pdsb[:])

                m8 = scd.tile([128, 8], F32, tag="m8")
                idx = scd.tile([128, 24], U32, tag="idx")
                nc.vector.max(out=m8[:], in_=pdsb[:])
                nc.vector.max_index(idx[:, 0:8], m8[:], pdsb[:])
                nc.vector.match_replace(pdsb[:], m8[:], pdsb[:], NEG)
                nc.vector.max(out=m8[:], in_=pdsb[:])
                nc.vector.max_index(idx[:, 8:16], m8[:], pdsb[:])
                nc.vector.match_replace(pdsb[:], m8[:], pdsb[:], NEG)
                nc.vector.max(out=m8[:], in_=pdsb[:])
                nc.vector.max_index(idx[:, 16:24], m8[:], pdsb[:])
                if debug_taps and li == 0 and m == 0:
                    nc.sync.dma_start(out=dbg["dbg_idx"][:], in_=idx[:])

                nbr = scd.tile([128, K, O], F32, tag="nbr")
                for k in range(K):
                    nc.gpsimd.indirect_dma_start(
                        out=nbr[:, k, :], out_offset=None, in_=p_dram[li][:],
                        in_offset=bass.IndirectOffsetOnAxis(ap=idx[:, k:k + 1], axis=0),
                    )
                if debug_taps and li == 0 and m == 0:
                    nc.sync.dma_start(out=dbg["dbg_nbr"][:], in_=nbr[:].rearrange("p k o -> p (k o)"))
                red = scd.tile([128, O], F32, tag="red")
                nc.vector.tensor_reduce(
                    out=red[:], in_=nbr[:].rearrange("p k o -> p o k"),
                    axis=mybir.AxisListType.X, op=mybir.AluOpType.max,
                )
                nc.vector.tensor_add(red[:], red[:], qsb[:, m * O:(m + 1) * O])
                if debug_taps and li == 0 and m == 0:
                    nc.sync.dma_start(out=dbg["dbg_red"][:], in_=red[:])
                    nc.sync.dma_start(out=dbg["dbg_q"][:], in_=qsb[:, 0:O])
                xpt = scd.tile([128, O], F32, tag="xpt")
                nc.vector.scalar_tensor_tensor(
                    out=xpt[:], in0=red[:], scalar=0.2, in1=red[:],
                    op0=mybir.AluOpType.mult, op1=mybir.AluOpType.max)

                # transpose back to channel-major into catT
                # destination rows for this conv's output channels:
                #   conv0 -> catT0[0:64], conv1 -> catT0[64:128],
                #   conv2 -> catT1[0:128], conv3 -> catT2+catT3
                for oh in range((O + 127) // 128):
                    w = min(128, O - oh * 128)
                    tps = psum_sm.tile([128, 128], F32, tag="tp", bufs=1)
                    nc.tensor.transpose(tps[0:w, :], xpt[:, oh * 128:oh * 128 + w], identsb[:])
                    dst = catT[li + oh][0:w, blk]
                    nc.scalar.copy(dst, tps[0:w, :])

            if li < 3:
                curXT = catT[li][:]

        conv_stack.close()

        # ---------------- h5 + pooling ----------------
        head_stack = ExitStack()
        hs = head_stack.enter_context(tc.tile_pool(name="head_sbuf", bufs=1))
        hsd = head_stack.enter_context(tc.tile_pool(name="head_dbl", bufs=2))
        hps = head_stack.enter_context(tc.tile_pool(name="head_psum", bufs=2, space="PSUM"))

        w5sb = []
        catR = []
        CATROWS = [64, 64, 128, 128, 128]
        r0 = 0
        for kt, r in enumerate(CATROWS):
            t = hs.tile([r, 1024], F32R, tag=f"w5_{kt}", name=f"w5sb{kt}")
            nc.sync.dma_start(out=t[:], in_=w5[r0:r0 + r, :])
            w5sb.append(t)
            r0 += r
            cr = hs.tile([r, N], F32R, tag=f"catr_{kt}", name=f"catr{kt}")
            nc.scalar.activation(out=cr[:], in_=catT[kt][0:r, :], func=AF.Copy)
            catR.append(cr)
        s5sb, b5sb = [], []
        for mt in range(8):
            t = hs.tile([128, 1], F32, tag=f"s5_{mt}", name=f"s5sb{mt}")
            nc.sync.dma_start(out=t[:], in_=s5[mt * 128:(mt + 1) * 128].rearrange("(p one) -> p one", one=1))
            s5sb.append(t)
            t = hs.tile([128, 1], F32, tag=f"b5_{mt}", name=f"b5sb{mt}")
            nc.sync.dma_start(out=t[:], in_=b5[mt * 128:(mt + 1) * 128].rearrange("(p one) -> p one", one=1))
            b5sb.append(t)

        for mt in range(8):
            h5p = hps.tile([128, N], F32, tag="h5", bufs=1)
            for kt in range(5):
                for j in range(4):
                    cols = slice(j * 512, (j + 1) * 512)
                    nc.tensor.matmul(h5p[:, cols], w5sb[kt][:, mt * 128:(mt + 1) * 128],
                                     catR[kt][:, cols], start=(kt == 0), stop=(kt == 4))
            h5sb = hsd.tile([128, N], F32, tag="h5sb")
            nc.scalar.activation(out=h5sb[:], in_=h5p[:], func=AF.Identity,
                                 scale=s5sb[mt][:], bias=b5sb[mt][:])
            nc.vector.scalar_tensor_tensor(
                out=h5sb[:], in0=h5sb[:], scalar=0.2, in1=h5sb[:],
                op0=mybir.AluOpType.mult, op1=mybir.AluOpType.max)
            nc.vector.tensor_reduce(out=fsum[mt][:], in_=h5sb[:],
                                    axis=mybir.AxisListType.X, op=mybir.AluOpType.add)
            nc.vector.tensor_reduce(out=fmax[mt][:], in_=h5sb[:],
                                    axis=mybir.AxisListType.X, op=mybir.AluOpType.max)

        # ---------------- classifier head ----------------
        feat_tiles = fmax + fsum  # k-tile order: [max(1024); mean-sum(1024)]

        wl1sb = []
        for kt in range(16):
            t = hs.tile([128, 512], F32, tag=f"wl1_{kt}", name=f"wl1sb{kt}")
            nc.sync.dma_start(out=t[:], in_=wl1[kt * 128:(kt + 1) * 128, :])
            wl1sb.append(t)
        h6 = []
        for mt in range(4):
            hp = hps.tile([128, 1], F32, tag="hv")
            for kt in range(16):
                nc.tensor.matmul(hp[:], wl1sb[kt][:, mt * 128:(mt + 1) * 128],
                                 feat_tiles[kt][:], start=(kt == 0), stop=(kt == 15))
            bt = hs.tile([128, 1], F32, tag=f"b6_{mt}")
            nc.sync.dma_start(out=bt[:], in_=bias6[mt * 128:(mt + 1) * 128].rearrange("(p one) -> p one", one=1))
            hsb = hs.tile([128, 1], F32, tag=f"h6_{mt}")
            nc.scalar.activation(out=hsb[:], in_=hp[:], func=AF.Identity, bias=bt[:])
            nc.vector.scalar_tensor_tensor(
                out=hsb[:], in0=hsb[:], scalar=0.2, in1=hsb[:],
                op0=mybir.AluOpType.mult, op1=mybir.AluOpType.max)
            h6.append(hsb)

        wl2sb = []
        for kt in range(4):
            t = hs.tile([128, 256], F32, tag=f"wl2_{kt}", name=f"wl2sb{kt}")
            nc.sync.dma_start(out=t[:], in_=wl2[kt * 128:(kt + 1) * 128, :])
            wl2sb.append(t)
        h7 = []
        for mt in range(2):
            hp = hps.tile([128, 1], F32, tag="hv")
            for kt in range(4):
                nc.tensor.matmul(hp[:], wl2sb[kt][:, mt * 128:(mt + 1) * 128],
                                 h6[kt][:], start=(kt == 0), stop=(kt == 3))
            bt = hs.tile([128, 1], F32, tag=f"b7_{mt}")
            nc.sync.dma_start(out=bt[:], in_=bias2[mt * 128:(mt + 1) * 128].rearrange("(p one) -> p one", one=1))
            hsb = hs.tile([128, 1], F32, tag=f"h7_{mt}")
            nc.scalar.activation(out=hsb[:], in_=hp[:], func=AF.Identity, bias=bt[:])
            nc.vector.scalar_tensor_tensor(
                out=hsb[:], in0=hsb[:], scalar=0.2, in1=hsb[:],
                op0=mybir.AluOpType.mult, op1=mybir.AluOpType.max)
            h7.append(hsb)

        wl3sb = []
        for kt in range(2):
            t = hs.tile([128, 40], F32, tag=f"wl3_{kt}", name=f"wl3sb{kt}")
            nc.sync.dma_start(out=t[:], in_=wl3[kt * 128:(kt + 1) * 128, :])
            wl3sb.append(t)
        op = hps.tile([40, 1], F32, tag="out", bufs=1)
        for kt in range(2):
            nc.tensor.matmul(op[:], wl3sb[kt][:], h7[kt][:], start=(kt == 0), stop=(kt == 1))
        b3 = hs.tile([40, 1], F32, tag="b3")
        nc.sync.dma_start(out=b3[:], in_=bias3[:].rearrange("(p one) -> p one", one=1))
        osb = hs.tile([40, 1], F32, tag="osb")
        nc.vector.tensor_add(osb[:], op[:], b3[:])
        nc.sync.dma_start(out=out[:].rearrange("(p one) -> p one", one=1), in_=osb[:])

        head_stack.close()
        persist_stack.close()
        dram_pool_stack.close()

    nc.compile()
    return nc


def host_prep(inputs):
    """Fold BN into conv weights; build the shared (weight) part of in_maps."""
    f = np.float32
    d = {}
    for li, (C, O) in enumerate(CONVS):
        W = np.asarray(inputs[f"W{li + 1}"], f)
        g = np.asarray(inputs[f"g{li + 1}"], f)
        b = np.asarray(inputs[f"b{li + 1}"], f)
        s = (np.float32(BN_INV) * g).astype(f)
        wn = (W[:, :C] * s[:, None]).astype(f)
        wcn = ((W[:, C:] - W[:, :C]) * s[:, None]).astype(f)
        d[f"wn{li}"] = np.ascontiguousarray(wn.T)
        d[f"wcn{li}"] = np.ascontiguousarray(wcn.T)
        d[f"qb{li}"] = np.ascontiguousarray(-b[None, :])
    w5t = np.ascontiguousarray(np.asarray(inputs["W5"], f).T).copy()
    wb = w5t.view(np.uint32)
    wb &= np.uint32(0xFFFFF000)
    d["w5"] = w5t
    d["s5"] = (np.float32(BN_INV) * np.asarray(inputs["g5"], f)).astype(f)
    d["b5"] = np.asarray(inputs["b5"], f)
    s6 = (np.float32(BN_INV) * np.asarray(inputs["g6"], f)).astype(f)
    wl1 = (np.asarray(inputs["Wl1"], f) * s6[:, None]).astype(f)
    wl1[:, 1024:] *= np.float32(1.0 / 2048.0)
    d["wl1"] = np.ascontiguousarray(wl1.T)
    d["bias6"] = np.asarray(inputs["b6"], f)
    s7 = (np.float32(BN_INV) * np.asarray(inputs["g7"], f)).astype(f)
    d["wl2"] = np.ascontiguousarray((np.asarray(inputs["Wl2"], f) * s7[:, None]).T)
    d["bias2"] = (s7 * np.asarray(inputs["bl2"], f) + np.asarray(inputs["b7"], f)).astype(f)
    d["wl3"] = np.ascontiguousarray(np.asarray(inputs["Wl3"], f).T)
    d["bias3"] = np.asarray(inputs["bl3"], f)
    d["ident"] = np.eye(128, dtype=f)
    d["onesrow"] = np.ones((1, N), f)
    d["negonesrow"] = -np.ones((1, N), f)
    return d


def kernel(**inputs):
    if "nc" not in _CACHE:
        _CACHE["nc"] = build_nc()
    nc = _CACHE["nc"]
    shared = host_prep(inputs)
    points = np.asarray(inputs["points"], np.float32)
    B = points.shape[0]
    in_maps = []
    for c in range(B):
        m = dict(shared)
        m["xT"] = np.ascontiguousarray(points[c].reshape(3, N))
        in_maps.append(m)
    res = run_bass_kernel_spmd(nc, in_maps, list(range(B))).results
    return np.stack([res[c]["out"] for c in range(B)]).astype(np.float32)


if __name__ == "__main__":
    inputs = dict(np.load("/root/problem/inputs.npz"))
    out = kernel(**inputs)
    exp = np.load("/root/problem/expected.npy")
    err = np.abs(out - exp).max()
    print("absmax diff:", err, "rel:", err / np.abs(exp).max())

